# revision 66
# baseline (speedup 1.0000x reference)
"""DAG-aware masked attention on 8 Trainium2 NeuronCores — v2.

Model: B=2, S=4096, DM=512, H=8 heads, DK=64.
  q/k/v = x @ W^T + b ; scores = (q k^T)/sqrt(DK) masked by dag_mask;
  out = softmax(scores) @ v ; y = out @ wo^T + bo

Sharding (data + sequence parallel, zero cross-core comms):
  core c -> batch b = c//4, query slice j = c%4 (1024 rows of S).

Design (measured 389us on HW, vs 432-499us baseline):
  - The scalar engine is the hard floor: 256 exp ACTIVATEs x ~1.15us
    (33.5M score elements/core at 1 elem/cycle/lane) = 293us busy.  The
    whole kernel is organized to keep that stream gapless.
  - All large inputs (x, weights, mask) are converted to bf16 on the host
    and DMA'd straight into their SBUF layout: no staging tiles, no device
    CAST ops, half the HBM traffic.
  - Attention processes heads in PAIRS (2j, 2j+1): QK for both heads of a
    pair contract over disjoint partition ranges (0-63 / 64-127) of the
    same kT/qT channel.  Head 2j's masked-exp tiles feed its AV chain
    immediately; head 2j+1's persist in SBUF, and its AV chain + both
    accumulator adds are pushed onto a filler FIFO.
  - The filler FIFO holds ALL deferrable PE work (second-head AV chains,
    K/Q/V projections for later blocks, prefetch DMAs, bias folding) as
    1-4 instruction units, drained 1-2 units after each key chunk's QK so
    PE-side work never clumps ahead of the next scores and the exp stream
    never starves.  drain_open() guarantees the previous pair's psAV slot
    rotation happens before the next pair's accumulator is allocated.
  - PSUM: 3 score slots (4KB each, shared by projections one-at-a-time)
    + 1 AV accumulator slot (4KB) = exactly 8 banks.
  - V carries a ones-column so the AV matmul yields the softmax
    denominator for free; softmax skips max-subtraction (|score| bounded);
    the dag mask is a bf16 multiply after exp, in place.
  - kb3 streams per-HEAD denominator reciprocals / normalization through
    the filler queue as each head's accumulator add lands, so only a short
    dense output projection (bias added on DVE) remains at the end.
  - The output store is bf16 (host casts back to f32): the final 8 chunk
    DMAs sit on the critical path before NEFF teardown, and halving the
    bytes consistently saves ~2-3us.
  - v2.1 (measured 377-389us, mean ~381): (1) all of block kb+1's K/V
    projection units are queued by the end of pair 1, so the filler FIFO
    runs dry before each block boundary and the next block's first QK is
    never stuck behind a clump of projection matmuls in the in-order PE
    queue; (2) head-h0's AV matmuls trail their exp by AVD=4 key chunks,
    so they never block the PE queue head on the psAV write-after-read
    hazard against the previous pair's not-yet-drained accumulator add
    (this was a ~3.4us exp-stream stall at every pair boundary); (3) 40
    junk matmuls on one 8-range-rotating PSUM tile trip the PE's HAM
    clock gate to 8/8 during the initial DMA wait; (4) the output
    projection is split per chunk into a dc0-2 PSUM-held prefix (emitted
    after the last pair's AV units drain, filling the reciprocal-chain
    window) and a dc3 finisher that alone waits the final normalization.
"""

import sys
import os

for _p in ("/root/.axon_site/_ro/trn_rl_repo", "/opt/trn_rl_repo"):
    if os.path.isdir(_p) and _p not in sys.path:
        sys.path.append(_p)

import numpy as np
import ml_dtypes

import concourse.bass as bass
import concourse.bacc as bacc
import concourse.tile as tile
import concourse.mybir as mybir
from concourse.bass_utils import run_bass_kernel_spmd

F32 = mybir.dt.float32
BF16 = mybir.dt.bfloat16
AF = mybir.ActivationFunctionType
NPBF16 = ml_dtypes.bfloat16


# ---------------------------------------------------------------------------
# Problem constants (hardcoded per the harness contract)
# ---------------------------------------------------------------------------
B, S, DM, H = 2, 4096, 512, 8
DK = DM // H          # 64
P = 128               # SBUF partitions
NCORES = 8
SLOC = 1024           # query rows per core
NKB = S // 1024       # 4 key blocks
KBS = 1024            # keys per block
NKC = KBS // P        # 8 key chunks (of 128) per block
QTS = 512             # query tile (PSUM bank = 512 f32)
NQT = SLOC // QTS     # 2
DCH = DM // P         # 4 feature chunks
NPAIR = H // 2        # 4 head pairs

_CACHED_NC = None


def _build_program():
    nc = bacc.Bacc("TRN2", target_bir_lowering=False, debug=False,
                   num_devices=NCORES)

    xT = nc.dram_tensor("xT", [DM, S], BF16, kind="ExternalInput").ap()
    maskT = nc.dram_tensor("maskT", [S, SLOC], BF16, kind="ExternalInput").ap()
    w_dram = {}
    b_dram = {}
    for name in ("wq", "wk", "wv", "wo"):
        w_dram[name] = nc.dram_tensor(name + "T", [DM, DM], BF16,
                                      kind="ExternalInput").ap()
    for name in ("bq", "bk", "bv", "bo"):
        b_dram[name] = nc.dram_tensor(name, [DM], F32,
                                      kind="ExternalInput").ap()
    out = nc.dram_tensor("out", [DM, SLOC], BF16, kind="ExternalOutput").ap()

    from contextlib import ExitStack
    with tile.TileContext(nc) as tc:
        with ExitStack() as ctx:
            pool = lambda **kw: ctx.enter_context(tc.tile_pool(**kw))
            wconst = pool(name="wconst", bufs=1)
            xbp = pool(name="xbp", bufs=2)
            kvp = pool(name="kvp", bufs=2)
            maskp = pool(name="maskp", bufs=2)
            pt0p = pool(name="pt0p", bufs=8)
            pt1p = pool(name="pt1p", bufs=12)
            accp = pool(name="accp", bufs=1)
            finp = pool(name="finp", bufs=1)
            rbp = pool(name="rbp", bufs=4)
            oep = pool(name="oep", bufs=2)
            # 6 banks of score tiles (3-deep pipeline) + 2 banks AV
            # accumulator = all 8 PSUM banks; projections borrow score slots
            psS = pool(name="psS", bufs=3, space="PSUM")
            psP = psS
            psAV = pool(name="psAV", bufs=1, space="PSUM")
            dramp = pool(name="dramp", bufs=1, space="DRAM")

            # ---- weights + biases straight to SBUF (already bf16) ----
            w_sb = {}
            b_sb = {}

            def emit_weight(name):
                wsb = wconst.tile([P, DCH, DM], BF16, tag=name,
                                  name=f"wsb_{name}")
                nc.gpsimd.dma_start(
                    out=wsb[:],
                    in_=w_dram[name].rearrange("(dc p) e -> p dc e", p=P))
                w_sb[name] = wsb

            def emit_bias(name):
                bt = wconst.tile([P, DCH], F32, tag=name, name=f"bt_{name}")
                nc.gpsimd.dma_start(
                    out=bt[:], in_=b_dram[name].rearrange("(c p) -> p c", p=P))
                b_sb[name] = bt

            # Q^T for this core's 1024 queries (filled during kb == 0)
            qT = wconst.tile([P, DCH, SLOC], BF16, tag="qT")
            # AV'^T accumulators, one per head: rows 0..63 = sum pm*V,
            # row 64 = softmax denominator l.
            avacc = [accp.tile([DK + 1, SLOC], F32, tag=f"av{h}",
                               name=f"avacc{h}") for h in range(H)]

            # per-block tiles, produced by prefetch units
            st_xb = {}
            st_kT = {}
            st_v = {}
            st_m = {}

            def emit_load_x(kb):
                xb = xbp.tile([P, DCH, KBS], BF16, tag="xb", name=f"xb{kb}")
                for dc in range(DCH):
                    nc.sync.dma_start(
                        out=xb[:, dc, :],
                        in_=xT[dc * P:(dc + 1) * P, kb * KBS:(kb + 1) * KBS])
                st_xb[kb] = xb

            def emit_kproj_unit(kb, ec, q2):
                if kb not in st_kT:
                    st_kT[kb] = kvp.tile([P, DCH, KBS], BF16, tag="kT",
                                         name=f"kT{kb}")
                kT = st_kT[kb]
                xb = st_xb[kb]
                # one PSUM slot held at a time (slots shared with scores)
                kps = psP.tile([P, QTS], F32, tag="s",
                               name=f"kps{kb}_{ec}_{q2}")
                for dc in range(DCH):
                    nc.tensor.matmul(
                        kps[:],
                        w_sb["wk"][:, dc, ec * P:(ec + 1) * P],
                        xb[:, dc, q2 * QTS:(q2 + 1) * QTS],
                        start=(dc == 0), stop=(dc == DCH - 1))
                nc.vector.tensor_scalar_add(
                    kT[:, ec, q2 * QTS:(q2 + 1) * QTS], kps[:],
                    b_sb["bk"][:, ec:ec + 1])

            def emit_qproj_unit(ec, q2):
                xb = st_xb[0]
                qps = psP.tile([P, QTS], F32, tag="s",
                               name=f"qps{ec}_{q2}")
                for dc in range(DCH):
                    nc.tensor.matmul(
                        qps[:],
                        w_sb["wq"][:, dc, ec * P:(ec + 1) * P],
                        xb[:, dc, q2 * QTS:(q2 + 1) * QTS],
                        start=(dc == 0), stop=(dc == DCH - 1))
                nc.vector.tensor_scalar_add(
                    qT[:, ec, q2 * QTS:(q2 + 1) * QTS], qps[:],
                    b_sb["bq"][:, ec:ec + 1])

            def emit_vproj(kb, scs):
                if kb not in st_v:
                    v = kvp.tile([P, NKC, H, DK + 1], BF16, tag="v",
                                 name=f"v{kb}")
                    nc.gpsimd.memset(v[:, :, :, DK:DK + 1], 1.0)
                    st_v[kb] = v
                vsb = st_v[kb]
                xb = st_xb[kb]
                for sc in scs:
                    vps = psP.tile([P, DM], F32, tag="s",
                                   name=f"vps{kb}_{sc}")
                    for dc in range(DCH):
                        nc.tensor.matmul(
                            vps[:],
                            xb[:, dc, sc * P:(sc + 1) * P],
                            w_sb["wv"][:, dc, :],
                            start=(dc == 0), stop=(dc == DCH - 1))
                    nc.vector.tensor_copy(
                        vsb[:, sc, :, 0:DK],
                        vps.rearrange("p (h e) -> p h e", h=H))

            def emit_mask(kb, kcs):
                if kb not in st_m:
                    st_m[kb] = maskp.tile([P, NKC, SLOC], BF16, tag="m",
                                          name=f"m{kb}")
                msb = st_m[kb]
                for kc in kcs:
                    nc.sync.dma_start(
                        out=msb[:, kc, :],
                        in_=maskT[kb * KBS + kc * P:kb * KBS + (kc + 1) * P, :])

            # l rows bounce through DRAM and come back fanned out over
            # 64 lanes so the iterative-divide reciprocal runs on 64 lanes
            l_dram = dramp.tile([H, SLOC], F32, name="l_dram")
            r_dram = dramp.tile([H, SLOC], F32, name="r_dram")
            onorm = finp.tile([P, DCH, SLOC], BF16, tag="onorm")

            rsdk = float(1.0 / np.sqrt(DK))

            # FIFO of deferrable work units, drained a little after each
            # key chunk so PE-side work (AV bursts, projections, prefetch)
            # never clumps and starves the scalar exp stream.
            # Each unit: (uses_psum_slot, emit_fn).
            filler = []

            def drain(npe=2, npsum=1, ndma=2):
                pe = ps = dm = 0
                i = 0
                while i < len(filler):
                    psum, dma, fn = filler[i][:3]
                    if dma:
                        if dm >= ndma:
                            i += 1
                            continue
                        filler.pop(i)
                        fn()
                        dm += 1
                        continue
                    if psum:
                        if ps >= npsum:
                            i += 1
                            continue
                        filler.pop(i)
                        fn()
                        ps += 1
                        continue
                    if pe >= npe:
                        i += 1
                        continue
                    filler.pop(i)
                    fn()
                    pe += 1

            def drain_all():
                while filler:
                    filler.pop(0)[2]()

            def drain_open():
                # pop the FIFO prefix through the previous pair's u_open
                # (if queued) so the psAV slot rotation stays in order
                k = next((i for i, u in enumerate(filler)
                          if len(u) > 3 and u[3] == "open"), None)
                if k is not None:
                    for _ in range(k + 1):
                        filler.pop(0)[2]()

            def push_kproj(kb, ec):
                for q2 in range(KBS // QTS):
                    filler.append((True, False,
                                   lambda kb=kb, ec=ec, q2=q2:
                                   emit_kproj_unit(kb, ec, q2)))

            def push_qproj(ec):
                for q2 in range(NQT):
                    filler.append((True, False,
                                   lambda ec=ec, q2=q2:
                                   emit_qproj_unit(ec, q2)))

            def push_vproj(kb, scs):
                for sc in scs:
                    filler.append((True, False,
                                   lambda kb=kb, sc=sc:
                                   emit_vproj(kb, [sc])))

            def push_dma(fn):
                filler.append((False, True, fn))

            def emit_half_tail(j, hh):
                """kb3: one head's softmax denominator -> reciprocal ->
                normalized attention rows (overlaps later pairs)."""
                h = 2 * j + hh
                po = hh * DK
                lp = finp.tile([8, SLOC // 8], F32, tag=f"lp{h}",
                               name=f"lp{h}")
                nc.sync.dma_start(
                    out=lp[:],
                    in_=l_dram[h:h + 1]
                    .rearrange("h (a b) -> (h a) b", a=8))
                rp = finp.tile([8, SLOC // 8], F32, tag=f"rp{h}",
                               name=f"rp{h}")
                nc.vector.reciprocal(rp[:], lp[:])
                nc.sync.dma_start(
                    out=r_dram[h:h + 1]
                    .rearrange("h (a b) -> (h a) b", a=8),
                    in_=rp[:])
                for qt in range(NQT):
                    qsl = slice(qt * QTS, (qt + 1) * QTS)
                    rb = rbp.tile([DK, QTS], F32, tag="rb",
                                  name=f"rb{h}_{qt}")
                    nc.sync.dma_start(
                        out=rb[:],
                        in_=r_dram[h:h + 1, qsl]
                        .to_broadcast((DK, QTS)))
                    nc.vector.tensor_mul(
                        onorm[po:po + DK, j, qsl],
                        avacc[h][0:DK, qsl], rb[:])

            AVD = 4  # AV-h0 lag in key chunks

            def emit_attention(kb):
                """Attention over key block kb, head-pair at a time; the
                second head's AV chain + accumulator adds are pushed as
                filler units that drip through the next pair's stream."""
                last = (kb == NKB - 1)
                for j in range(NPAIR):
                    h0, h1 = 2 * j, 2 * j + 1
                    stv = {}
                    pt0s = []
                    pt1s = []

                    def emit_avh0(k, kb=kb, h0=h0, stv=stv, pt0s=pt0s):
                        for qt in range(NQT):
                            qsl = slice(qt * QTS, (qt + 1) * QTS)
                            nc.tensor.matmul(
                                stv["avps0"][:, qsl],
                                st_v[kb][:, k, h0, :], pt0s[k][:, qsl],
                                start=(k == 0), stop=(k == NKC - 1))

                    for kc in range(NKC):
                        kT, msb = st_kT[kb], st_m[kb]
                        sp0 = psS.tile([P, SLOC], F32, tag="s",
                                       name=f"sp{kb}_{j}_{kc}_0")
                        sp1 = psS.tile([P, SLOC], F32, tag="s",
                                       name=f"sp{kb}_{j}_{kc}_1")
                        ksl = slice(kc * P, (kc + 1) * P)
                        for qt in range(NQT):
                            qsl = slice(qt * QTS, (qt + 1) * QTS)
                            nc.tensor.matmul(
                                sp0[:, qsl], kT[0:DK, j, ksl],
                                qT[0:DK, j, qsl], start=True, stop=True)
                            nc.tensor.matmul(
                                sp1[:, qsl], kT[DK:P, j, ksl],
                                qT[DK:P, j, qsl], start=True, stop=True)
                        if kc > 0:
                            drain()
                        # head h0: exp -> mask (in place) -> AV right away
                        pt0 = pt0p.tile([P, SLOC], BF16, tag="pt0",
                                        name=f"pt{kb}_{j}_{kc}_0")
                        nc.scalar.activation(pt0[:], sp0[:], AF.Exp,
                                             bias=0.0, scale=rsdk)
                        nc.vector.tensor_mul(pt0[:], pt0[:], msb[:, kc, :])
                        pt0s.append(pt0)
                        if kc == 0:
                            # the previous pair's u_open must rotate the
                            # psAV slot before this pair's accumulator
                            drain_open()
                            drain(npe=1)
                            stv["avps0"] = psAV.tile([DK + 1, SLOC], F32,
                                                     tag="av",
                                                     name=f"avps{kb}_{h0}")
                        # h0's AV matmuls trail AVD chunks behind so they
                        # never sit blocked at the PE queue head on the
                        # psAV WAR against the previous pair's u_close
                        if kc >= AVD:
                            emit_avh0(kc - AVD)
                        if kc > 0:
                            drain(npe=1, npsum=0, ndma=1)
                        # head h1: exp -> mask; P tile persists, its AV
                        # chain drips through the next pair's stream
                        pt1 = pt1p.tile([P, SLOC], BF16, tag="pt1",
                                        name=f"pt{kb}_{j}_{kc}_1")
                        nc.scalar.activation(pt1[:], sp1[:], AF.Exp,
                                             bias=0.0, scale=rsdk)
                        nc.vector.tensor_mul(pt1[:], pt1[:], msb[:, kc, :])
                        pt1s.append(pt1)
                    for k in range(NKC - AVD, NKC):
                        filler.append((False, False,
                                       lambda k=k, f=emit_avh0: f(k)))

                    # ---- push the pair's wrap-up as fine-grained units ----
                    st = {}

                    def u_open(kb=kb, j=j, h0=h0, h1=h1, stv=stv,
                               pt1s=pt1s, last=last, st=st):
                        if kb == 0:
                            nc.vector.tensor_copy(avacc[h0][:],
                                                  stv["avps0"][:])
                        else:
                            nc.vector.tensor_add(avacc[h0][:], avacc[h0][:],
                                                 stv["avps0"][:])
                        if last:
                            nc.sync.dma_start(out=l_dram[h0:h0 + 1, :],
                                              in_=avacc[h0][DK:DK + 1, :])
                            emit_half_tail(j, 0)
                        st["avps1"] = psAV.tile([DK + 1, SLOC], F32,
                                                tag="av",
                                                name=f"avps{kb}_{h1}")

                    def u_av(kc, kb=kb, h1=h1, pt1s=pt1s, st=st):
                        for qt in range(NQT):
                            qsl = slice(qt * QTS, (qt + 1) * QTS)
                            nc.tensor.matmul(
                                st["avps1"][:, qsl], st_v[kb][:, kc, h1, :],
                                pt1s[kc][:, qsl],
                                start=(kc == 0), stop=(kc == NKC - 1))

                    def u_close(kb=kb, j=j, h1=h1, last=last, st=st):
                        if kb == 0:
                            nc.vector.tensor_copy(avacc[h1][:],
                                                  st["avps1"][:])
                        else:
                            nc.vector.tensor_add(avacc[h1][:], avacc[h1][:],
                                                 st["avps1"][:])
                        if last:
                            nc.sync.dma_start(out=l_dram[h1:h1 + 1, :],
                                              in_=avacc[h1][DK:DK + 1, :])
                            emit_half_tail(j, 1)

                    filler.append((False, False, u_open, "open"))
                    for kc in range(NKC):
                        filler.append((False, False,
                                       lambda kc=kc: u_av(kc)))
                    filler.append((False, False, u_close, "close"))
                    push_prefetch(kb, j)

            # ---------------- program ----------------
            # bo2 = bo + wo^T-contraction of bv, via filler units
            # (folds the V bias into the output-projection bias)
            bvb = wconst.tile([P, DCH], BF16, tag="bvb")
            bo2 = wconst.tile([P, DCH], F32, tag="bo2")

            def u_bo2(ec):
                if ec == 0:
                    nc.vector.tensor_copy(bvb[:], b_sb["bv"][:])
                bps = psP.tile([P, 1], F32, tag="s", name=f"bps{ec}")
                for dc in range(DCH):
                    nc.tensor.matmul(
                        bps[:], w_sb["wo"][:, dc, ec * P:(ec + 1) * P],
                        bvb[:, dc:dc + 1],
                        start=(dc == 0), stop=(dc == DCH - 1))
                nc.vector.tensor_scalar_add(bo2[:, ec:ec + 1], bps[:],
                                            b_sb["bo"][:, ec:ec + 1])

            # Per-pair prefetch pushes, balanced to the 8 psum-unit slots
            # each pair's drains provide (see drain() budgets).
            def push_prefetch(kb, j):
                if kb == 0:
                    if j == 0:
                        push_kproj(0, 2)
                        push_qproj(2)
                    elif j == 1:
                        push_kproj(0, 3)
                        push_qproj(3)
                elif kb == 1 and j == 1:
                    for ec in range(DCH):
                        filler.append((True, False,
                                       lambda ec=ec: u_bo2(ec)))
                # everything for kb+1 is queued by the end of pair 1 so
                # the filler runs dry before the block boundary and the
                # next block's first QK isn't stuck behind a clump of
                # projection matmuls in the PE queue
                if kb + 1 < NKB:
                    n = kb + 1
                    if j == 0:
                        push_dma(lambda n=n: emit_load_x(n))
                        push_dma(lambda n=n: emit_mask(n, range(0, 4)))
                        push_dma(lambda n=n: emit_mask(n, range(4, NKC)))
                        for ec in range(DCH):
                            push_kproj(n, ec)
                    elif j == 1:
                        push_vproj(n, range(NKC))

            # Upfront: just enough for pair 0 of kb0 to start immediately.
            # Junk matmuls (one tile, 8 rotating ranges, so the WAW deps
            # are deep enough to stream) trip the PE's HAM clock gate to
            # 8/8 during the x/weight DMA wait: the first projections
            # then run at 2.4GHz instead of 1.2.
            warm = wconst.tile([P, P], BF16, tag="warm")
            nc.gpsimd.memset(warm[:], 0.0)
            wps = psS.tile([P, SLOC], F32, tag="s", name="warmps")
            for i in range(40):
                c = (i % 8) * P
                nc.tensor.matmul(wps[:, c:c + P], warm[:], warm[:],
                                 start=True, stop=True)
            emit_load_x(0)
            emit_weight("wk")
            emit_bias("bk")
            emit_weight("wq")
            emit_bias("bq")
            emit_weight("wv")
            emit_bias("bv")
            for q2 in range(NQT):
                emit_kproj_unit(0, 0, q2)
            for q2 in range(NQT):
                emit_qproj_unit(0, q2)
            emit_vproj(0, [0, 1, 2, 3])
            emit_mask(0, range(NKC))
            emit_weight("wo")
            emit_bias("bo")
            # interleave pair-1's K/Q channel with kb0's remaining V chunks
            filler.append((True, False, lambda: emit_vproj(0, [4])))
            filler.append((True, False, lambda: emit_kproj_unit(0, 1, 0)))
            filler.append((True, False, lambda: emit_kproj_unit(0, 1, 1)))
            filler.append((True, False, lambda: emit_vproj(0, [5])))
            filler.append((True, False, lambda: emit_qproj_unit(1, 0)))
            filler.append((True, False, lambda: emit_qproj_unit(1, 1)))
            filler.append((True, False, lambda: emit_vproj(0, [6])))
            filler.append((True, False, lambda: emit_vproj(0, [7])))

            for kb in range(NKB):
                emit_attention(kb)

            # ---- tail: dense output projection, split per chunk into a
            # dc0-2 PSUM-held prefix and a dc3 finisher (dc3 = last head
            # pair, the only part that waits the final normalization).
            # The last pair's AV units are drained FIRST so the prefix
            # matmuls don't delay its accumulator adds.
            def oproj_pre(qt, ec):
                ops = psS.tile([P, QTS], F32, tag="s",
                               name=f"ops{ec}_{qt}")
                for dc in range(DCH - 1):
                    nc.tensor.matmul(
                        ops[:],
                        w_sb["wo"][:, dc, ec * P:(ec + 1) * P],
                        onorm[:, dc, qt * QTS:(qt + 1) * QTS],
                        start=(dc == 0), stop=False)
                return ops

            def oproj_post(qt, ec, ops):
                dc = DCH - 1
                nc.tensor.matmul(
                    ops[:],
                    w_sb["wo"][:, dc, ec * P:(ec + 1) * P],
                    onorm[:, dc, qt * QTS:(qt + 1) * QTS],
                    start=False, stop=True)
                oev = oep.tile([P, QTS], BF16, tag="oev",
                               name=f"oev{ec}_{qt}")
                nc.vector.tensor_scalar_add(oev[:], ops[:],
                                            bo2[:, ec:ec + 1])
                nc.sync.dma_start(
                    out=out[ec * P:(ec + 1) * P, qt * QTS:(qt + 1) * QTS],
                    in_=oev[:])

            while filler and (len(filler[0]) <= 3 or
                              filler[0][3] != "close"):
                filler.pop(0)[2]()
            pres = [(qt, ec, oproj_pre(qt, ec))
                    for qt, ec in [(0, 0), (1, 0), (0, 1)]]
            drain_all()  # u_close: h7's accumulator add + tail
            for qt, ec, ops in pres:
                oproj_post(qt, ec, ops)
            for qt, ec in [(1, 1), (0, 2), (1, 2), (0, 3), (1, 3)]:
                oproj_post(qt, ec, oproj_pre(qt, ec))
    nc.compile()
    return nc


def get_program():
    global _CACHED_NC
    if _CACHED_NC is None:
        _CACHED_NC = _build_program()
    return _CACHED_NC


def make_in_maps(x, dag_mask, wq, bq, wk, bk, wv, bv, wo, bo):
    """Host-side sharding: slices/transposes/rotations + bf16 casts only."""
    shared = {
        "wqT": np.ascontiguousarray(wq.T).astype(NPBF16),
        "wkT": np.ascontiguousarray(wk.T).astype(NPBF16),
        "wvT": np.ascontiguousarray(wv.T).astype(NPBF16),
        "woT": np.ascontiguousarray(wo.T).astype(NPBF16),
        "bq": np.ascontiguousarray(bq), "bk": np.ascontiguousarray(bk),
        "bv": np.ascontiguousarray(bv), "bo": np.ascontiguousarray(bo),
    }
    xTs = [np.ascontiguousarray(x[b].T).astype(NPBF16) for b in range(B)]
    mask_bf = dag_mask.astype(NPBF16)
    in_maps = []
    for c in range(NCORES):
        b, j = divmod(c, NCORES // B)
        s0 = j * SLOC
        # rotate the key axis so program block 0 == this core's query slice
        xTb = xTs[b]
        xT_rot = np.ascontiguousarray(
            np.concatenate([xTb[:, s0:], xTb[:, :s0]], axis=1))
        mT = mask_bf[s0:s0 + SLOC, :].T  # (S keys, SLOC queries)
        mT_rot = np.ascontiguousarray(
            np.concatenate([mT[s0:, :], mT[:s0, :]], axis=0))
        in_maps.append({"xT": xT_rot, "maskT": mT_rot, **shared})
    return in_maps


def kernel(x, dag_mask, wq, bq, wk, bk, wv, bv, wo, bo, trace=False):
    x = np.asarray(x, dtype=np.float32)
    dag_mask = np.asarray(dag_mask, dtype=np.int32)
    args = [np.asarray(a, dtype=np.float32)
            for a in (wq, bq, wk, bk, wv, bv, wo, bo)]
    nc = get_program()
    in_maps = make_in_maps(x, dag_mask, *args)
    core_ids = list(range(NCORES))
    res = run_bass_kernel_spmd(nc, in_maps, core_ids, trace=trace)
    out = np.empty((B, S, DM), np.float32)
    for c in range(NCORES):
        b, j = divmod(c, NCORES // B)
        s0 = j * SLOC
        out[b, s0:s0 + SLOC, :] = res.results[c]["out"].T.astype(np.float32)
    if trace:
        return out, res
    return out



# revision 67
# speedup vs baseline: 1.0057x; 1.0057x over previous
"""DAG-aware masked attention on 8 Trainium2 NeuronCores — v2.

Model: B=2, S=4096, DM=512, H=8 heads, DK=64.
  q/k/v = x @ W^T + b ; scores = (q k^T)/sqrt(DK) masked by dag_mask;
  out = softmax(scores) @ v ; y = out @ wo^T + bo

Sharding (data + sequence parallel, zero cross-core comms):
  core c -> batch b = c//4, query slice j = c%4 (1024 rows of S).

Design (measured 389us on HW, vs 432-499us baseline):
  - The scalar engine is the hard floor: 256 exp ACTIVATEs x ~1.15us
    (33.5M score elements/core at 1 elem/cycle/lane) = 293us busy.  The
    whole kernel is organized to keep that stream gapless.
  - All large inputs (x, weights, mask) are converted to bf16 on the host
    and DMA'd straight into their SBUF layout: no staging tiles, no device
    CAST ops, half the HBM traffic.
  - Attention processes heads in PAIRS (2j, 2j+1): QK for both heads of a
    pair contract over disjoint partition ranges (0-63 / 64-127) of the
    same kT/qT channel.  Head 2j's masked-exp tiles feed its AV chain
    immediately; head 2j+1's persist in SBUF, and its AV chain + both
    accumulator adds are pushed onto a filler FIFO.
  - The filler FIFO holds ALL deferrable PE work (second-head AV chains,
    K/Q/V projections for later blocks, prefetch DMAs, bias folding) as
    1-4 instruction units, drained 1-2 units after each key chunk's QK so
    PE-side work never clumps ahead of the next scores and the exp stream
    never starves.  drain_open() guarantees the previous pair's psAV slot
    rotation happens before the next pair's accumulator is allocated.
  - PSUM: 3 score slots (4KB each, shared by projections one-at-a-time)
    + 1 AV accumulator slot (4KB) = exactly 8 banks.
  - V carries a ones-column so the AV matmul yields the softmax
    denominator for free; softmax skips max-subtraction (|score| bounded);
    the dag mask is a bf16 multiply after exp, in place.
  - kb3 streams per-HEAD denominator reciprocals / normalization through
    the filler queue as each head's accumulator add lands, so only a short
    dense output projection (bias added on DVE) remains at the end.
  - The output store is bf16 (host casts back to f32): the final 8 chunk
    DMAs sit on the critical path before NEFF teardown, and halving the
    bytes consistently saves ~2-3us.
  - v2.1 (measured 377-389us, mean ~381): (1) all of block kb+1's K/V
    projection units are queued by the end of pair 1, so the filler FIFO
    runs dry before each block boundary and the next block's first QK is
    never stuck behind a clump of projection matmuls in the in-order PE
    queue; (2) head-h0's AV matmuls trail their exp by AVD=4 key chunks,
    so they never block the PE queue head on the psAV write-after-read
    hazard against the previous pair's not-yet-drained accumulator add
    (this was a ~3.4us exp-stream stall at every pair boundary); (3) 40
    junk matmuls on one 8-range-rotating PSUM tile trip the PE's HAM
    clock gate to 8/8 during the initial DMA wait; (4) the output
    projection is split per chunk into a dc0-2 PSUM-held prefix (emitted
    after the last pair's AV units drain, filling the reciprocal-chain
    window) and a dc3 finisher that alone waits the final normalization.
"""

import sys
import os

for _p in ("/root/.axon_site/_ro/trn_rl_repo", "/opt/trn_rl_repo"):
    if os.path.isdir(_p) and _p not in sys.path:
        sys.path.append(_p)

import numpy as np
import ml_dtypes

import concourse.bass as bass
import concourse.bacc as bacc
import concourse.tile as tile
import concourse.mybir as mybir
from concourse.bass_utils import run_bass_kernel_spmd

F32 = mybir.dt.float32
BF16 = mybir.dt.bfloat16
AF = mybir.ActivationFunctionType
NPBF16 = ml_dtypes.bfloat16


# ---------------------------------------------------------------------------
# Problem constants (hardcoded per the harness contract)
# ---------------------------------------------------------------------------
B, S, DM, H = 2, 4096, 512, 8
DK = DM // H          # 64
P = 128               # SBUF partitions
NCORES = 8
SLOC = 1024           # query rows per core
NKB = S // 1024       # 4 key blocks
KBS = 1024            # keys per block
NKC = KBS // P        # 8 key chunks (of 128) per block
QTS = 512             # query tile (PSUM bank = 512 f32)
NQT = SLOC // QTS     # 2
DCH = DM // P         # 4 feature chunks
NPAIR = H // 2        # 4 head pairs

_CACHED_NC = None


def _build_program():
    nc = bacc.Bacc("TRN2", target_bir_lowering=False, debug=False,
                   num_devices=NCORES)

    xT = nc.dram_tensor("xT", [DM, S], BF16, kind="ExternalInput").ap()
    maskT = nc.dram_tensor("maskT", [S, SLOC], BF16, kind="ExternalInput").ap()
    w_dram = {}
    b_dram = {}
    for name in ("wq", "wk", "wv", "wo"):
        w_dram[name] = nc.dram_tensor(name + "T", [DM, DM], BF16,
                                      kind="ExternalInput").ap()
    for name in ("bq", "bk", "bv", "bo"):
        b_dram[name] = nc.dram_tensor(name, [DM], F32,
                                      kind="ExternalInput").ap()
    out = nc.dram_tensor("out", [DM, SLOC], BF16, kind="ExternalOutput").ap()

    from contextlib import ExitStack
    with tile.TileContext(nc) as tc:
        with ExitStack() as ctx:
            pool = lambda **kw: ctx.enter_context(tc.tile_pool(**kw))
            wconst = pool(name="wconst", bufs=1)
            xbp = pool(name="xbp", bufs=2)
            kvp = pool(name="kvp", bufs=2)
            maskp = pool(name="maskp", bufs=2)
            pt0p = pool(name="pt0p", bufs=8)
            pt1p = pool(name="pt1p", bufs=12)
            accp = pool(name="accp", bufs=1)
            finp = pool(name="finp", bufs=1)
            rbp = pool(name="rbp", bufs=4)
            oep = pool(name="oep", bufs=2)
            # 6 banks of score tiles (3-deep pipeline) + 2 banks AV
            # accumulator = all 8 PSUM banks; projections borrow score slots
            psS = pool(name="psS", bufs=3, space="PSUM")
            psP = psS
            psAV = pool(name="psAV", bufs=1, space="PSUM")
            dramp = pool(name="dramp", bufs=1, space="DRAM")

            # ---- weights + biases straight to SBUF (already bf16) ----
            w_sb = {}
            b_sb = {}

            def emit_weight(name):
                wsb = wconst.tile([P, DCH, DM], BF16, tag=name,
                                  name=f"wsb_{name}")
                nc.gpsimd.dma_start(
                    out=wsb[:],
                    in_=w_dram[name].rearrange("(dc p) e -> p dc e", p=P))
                w_sb[name] = wsb

            def emit_bias(name):
                bt = wconst.tile([P, DCH], F32, tag=name, name=f"bt_{name}")
                nc.gpsimd.dma_start(
                    out=bt[:], in_=b_dram[name].rearrange("(c p) -> p c", p=P))
                b_sb[name] = bt

            # Q^T for this core's 1024 queries (filled during kb == 0)
            qT = wconst.tile([P, DCH, SLOC], BF16, tag="qT")
            # AV'^T accumulators, one per head: rows 0..63 = sum pm*V,
            # row 64 = softmax denominator l.
            avacc = [accp.tile([DK + 1, SLOC], F32, tag=f"av{h}",
                               name=f"avacc{h}") for h in range(H)]

            # per-block tiles, produced by prefetch units
            st_xb = {}
            st_kT = {}
            st_v = {}
            st_m = {}

            def emit_load_x(kb):
                xb = xbp.tile([P, DCH, KBS], BF16, tag="xb", name=f"xb{kb}")
                for dc in range(DCH):
                    nc.sync.dma_start(
                        out=xb[:, dc, :],
                        in_=xT[dc * P:(dc + 1) * P, kb * KBS:(kb + 1) * KBS])
                st_xb[kb] = xb

            def emit_kproj_unit(kb, ec, q2):
                if kb not in st_kT:
                    st_kT[kb] = kvp.tile([P, DCH, KBS], BF16, tag="kT",
                                         name=f"kT{kb}")
                kT = st_kT[kb]
                xb = st_xb[kb]
                # one PSUM slot held at a time (slots shared with scores)
                kps = psP.tile([P, QTS], F32, tag="s",
                               name=f"kps{kb}_{ec}_{q2}")
                for dc in range(DCH):
                    nc.tensor.matmul(
                        kps[:],
                        w_sb["wk"][:, dc, ec * P:(ec + 1) * P],
                        xb[:, dc, q2 * QTS:(q2 + 1) * QTS],
                        start=(dc == 0), stop=(dc == DCH - 1))
                nc.vector.tensor_scalar_add(
                    kT[:, ec, q2 * QTS:(q2 + 1) * QTS], kps[:],
                    b_sb["bk"][:, ec:ec + 1])

            def emit_qproj_unit(ec, q2):
                xb = st_xb[0]
                qps = psP.tile([P, QTS], F32, tag="s",
                               name=f"qps{ec}_{q2}")
                for dc in range(DCH):
                    nc.tensor.matmul(
                        qps[:],
                        w_sb["wq"][:, dc, ec * P:(ec + 1) * P],
                        xb[:, dc, q2 * QTS:(q2 + 1) * QTS],
                        start=(dc == 0), stop=(dc == DCH - 1))
                nc.vector.tensor_scalar_add(
                    qT[:, ec, q2 * QTS:(q2 + 1) * QTS], qps[:],
                    b_sb["bq"][:, ec:ec + 1])

            def emit_vproj(kb, scs):
                if kb not in st_v:
                    v = kvp.tile([P, NKC, H, DK + 1], BF16, tag="v",
                                 name=f"v{kb}")
                    nc.gpsimd.memset(v[:, :, :, DK:DK + 1], 1.0)
                    st_v[kb] = v
                vsb = st_v[kb]
                xb = st_xb[kb]
                for sc in scs:
                    vps = psP.tile([P, DM], F32, tag="s",
                                   name=f"vps{kb}_{sc}")
                    for dc in range(DCH):
                        nc.tensor.matmul(
                            vps[:],
                            xb[:, dc, sc * P:(sc + 1) * P],
                            w_sb["wv"][:, dc, :],
                            start=(dc == 0), stop=(dc == DCH - 1))
                    nc.vector.tensor_copy(
                        vsb[:, sc, :, 0:DK],
                        vps.rearrange("p (h e) -> p h e", h=H))

            def emit_mask(kb, kcs):
                if kb not in st_m:
                    st_m[kb] = maskp.tile([P, NKC, SLOC], BF16, tag="m",
                                          name=f"m{kb}")
                msb = st_m[kb]
                for kc in kcs:
                    nc.sync.dma_start(
                        out=msb[:, kc, :],
                        in_=maskT[kb * KBS + kc * P:kb * KBS + (kc + 1) * P, :])

            # l rows bounce through DRAM and come back fanned out over
            # 64 lanes so the iterative-divide reciprocal runs on 64 lanes
            l_dram = dramp.tile([H, SLOC], F32, name="l_dram")
            r_dram = dramp.tile([H, SLOC], F32, name="r_dram")
            onorm = finp.tile([P, DCH, SLOC], BF16, tag="onorm")

            rsdk = float(1.0 / np.sqrt(DK))

            # FIFO of deferrable work units, drained a little after each
            # key chunk so PE-side work (AV bursts, projections, prefetch)
            # never clumps and starves the scalar exp stream.
            # Each unit: (uses_psum_slot, emit_fn).
            filler = []

            def drain(npe=2, npsum=1, ndma=2):
                pe = ps = dm = 0
                i = 0
                while i < len(filler):
                    psum, dma, fn = filler[i][:3]
                    if dma:
                        if dm >= ndma:
                            i += 1
                            continue
                        filler.pop(i)
                        fn()
                        dm += 1
                        continue
                    if psum:
                        if ps >= npsum:
                            i += 1
                            continue
                        filler.pop(i)
                        fn()
                        ps += 1
                        continue
                    if pe >= npe:
                        i += 1
                        continue
                    filler.pop(i)
                    fn()
                    pe += 1

            def drain_all():
                while filler:
                    filler.pop(0)[2]()

            def drain_open():
                # pop the FIFO prefix through the previous pair's u_open
                # (if queued) so the psAV slot rotation stays in order
                k = next((i for i, u in enumerate(filler)
                          if len(u) > 3 and u[3] == "open"), None)
                if k is not None:
                    for _ in range(k + 1):
                        filler.pop(0)[2]()

            def push_kproj(kb, ec):
                for q2 in range(KBS // QTS):
                    filler.append((True, False,
                                   lambda kb=kb, ec=ec, q2=q2:
                                   emit_kproj_unit(kb, ec, q2)))

            def push_qproj(ec):
                for q2 in range(NQT):
                    filler.append((True, False,
                                   lambda ec=ec, q2=q2:
                                   emit_qproj_unit(ec, q2)))

            def push_vproj(kb, scs):
                for sc in scs:
                    filler.append((True, False,
                                   lambda kb=kb, sc=sc:
                                   emit_vproj(kb, [sc])))

            def push_dma(fn):
                filler.append((False, True, fn))

            def emit_half_tail(j, hh):
                """kb3: one head's softmax denominator -> reciprocal ->
                normalized attention rows (overlaps later pairs)."""
                h = 2 * j + hh
                po = hh * DK
                lp = finp.tile([8, SLOC // 8], F32, tag=f"lp{h}",
                               name=f"lp{h}")
                nc.sync.dma_start(
                    out=lp[:],
                    in_=l_dram[h:h + 1]
                    .rearrange("h (a b) -> (h a) b", a=8))
                rp = finp.tile([8, SLOC // 8], F32, tag=f"rp{h}",
                               name=f"rp{h}")
                nc.vector.reciprocal(rp[:], lp[:])
                nc.sync.dma_start(
                    out=r_dram[h:h + 1]
                    .rearrange("h (a b) -> (h a) b", a=8),
                    in_=rp[:])
                for qt in range(NQT):
                    qsl = slice(qt * QTS, (qt + 1) * QTS)
                    rb = rbp.tile([DK, QTS], F32, tag="rb",
                                  name=f"rb{h}_{qt}")
                    nc.sync.dma_start(
                        out=rb[:],
                        in_=r_dram[h:h + 1, qsl]
                        .to_broadcast((DK, QTS)))
                    nc.vector.tensor_mul(
                        onorm[po:po + DK, j, qsl],
                        avacc[h][0:DK, qsl], rb[:])

            AVD = 4  # AV-h0 lag in key chunks

            def emit_attention(kb):
                """Attention over key block kb, head-pair at a time; the
                second head's AV chain + accumulator adds are pushed as
                filler units that drip through the next pair's stream."""
                last = (kb == NKB - 1)
                for j in range(NPAIR):
                    h0, h1 = 2 * j, 2 * j + 1
                    stv = {}
                    pt0s = []
                    pt1s = []

                    def emit_avh0(k, kb=kb, h0=h0, stv=stv, pt0s=pt0s):
                        for qt in range(NQT):
                            qsl = slice(qt * QTS, (qt + 1) * QTS)
                            nc.tensor.matmul(
                                stv["avps0"][:, qsl],
                                st_v[kb][:, k, h0, :], pt0s[k][:, qsl],
                                start=(k == 0), stop=(k == NKC - 1))

                    for kc in range(NKC):
                        kT, msb = st_kT[kb], st_m[kb]
                        sp0 = psS.tile([P, SLOC], F32, tag="s",
                                       name=f"sp{kb}_{j}_{kc}_0")
                        sp1 = psS.tile([P, SLOC], F32, tag="s",
                                       name=f"sp{kb}_{j}_{kc}_1")
                        ksl = slice(kc * P, (kc + 1) * P)
                        for qt in range(NQT):
                            qsl = slice(qt * QTS, (qt + 1) * QTS)
                            nc.tensor.matmul(
                                sp0[:, qsl], kT[0:DK, j, ksl],
                                qT[0:DK, j, qsl], start=True, stop=True)
                            nc.tensor.matmul(
                                sp1[:, qsl], kT[DK:P, j, ksl],
                                qT[DK:P, j, qsl], start=True, stop=True)
                        if kc > 0:
                            drain()
                        # head h0: exp -> mask (in place) -> AV right away
                        pt0 = pt0p.tile([P, SLOC], BF16, tag="pt0",
                                        name=f"pt{kb}_{j}_{kc}_0")
                        nc.scalar.activation(pt0[:], sp0[:], AF.Exp,
                                             bias=0.0, scale=rsdk)
                        nc.vector.tensor_mul(pt0[:], pt0[:], msb[:, kc, :])
                        pt0s.append(pt0)
                        if kc == 0:
                            # the previous pair's u_open must rotate the
                            # psAV slot before this pair's accumulator
                            drain_open()
                            drain(npe=1)
                            stv["avps0"] = psAV.tile([DK + 1, SLOC], F32,
                                                     tag="av",
                                                     name=f"avps{kb}_{h0}")
                        # h0's AV matmuls trail AVD chunks behind so they
                        # never sit blocked at the PE queue head on the
                        # psAV WAR against the previous pair's u_close
                        if kc >= AVD:
                            emit_avh0(kc - AVD)
                        if kc > 0:
                            drain(npe=1, npsum=0, ndma=1)
                        # head h1: exp -> mask; P tile persists, its AV
                        # chain drips through the next pair's stream
                        pt1 = pt1p.tile([P, SLOC], BF16, tag="pt1",
                                        name=f"pt{kb}_{j}_{kc}_1")
                        nc.scalar.activation(pt1[:], sp1[:], AF.Exp,
                                             bias=0.0, scale=rsdk)
                        nc.vector.tensor_mul(pt1[:], pt1[:], msb[:, kc, :])
                        pt1s.append(pt1)
                    for k in range(NKC - AVD, NKC):
                        filler.append((False, False,
                                       lambda k=k, f=emit_avh0: f(k)))

                    # ---- push the pair's wrap-up as fine-grained units ----
                    st = {}

                    def u_open(kb=kb, j=j, h0=h0, h1=h1, stv=stv,
                               pt1s=pt1s, last=last, st=st):
                        if kb == 0:
                            nc.vector.tensor_copy(avacc[h0][:],
                                                  stv["avps0"][:])
                        else:
                            nc.vector.tensor_add(avacc[h0][:], avacc[h0][:],
                                                 stv["avps0"][:])
                        if last:
                            nc.sync.dma_start(out=l_dram[h0:h0 + 1, :],
                                              in_=avacc[h0][DK:DK + 1, :])
                            emit_half_tail(j, 0)
                        st["avps1"] = psAV.tile([DK + 1, SLOC], F32,
                                                tag="av",
                                                name=f"avps{kb}_{h1}")

                    def u_av(kc, kb=kb, h1=h1, pt1s=pt1s, st=st):
                        for qt in range(NQT):
                            qsl = slice(qt * QTS, (qt + 1) * QTS)
                            nc.tensor.matmul(
                                st["avps1"][:, qsl], st_v[kb][:, kc, h1, :],
                                pt1s[kc][:, qsl],
                                start=(kc == 0), stop=(kc == NKC - 1))

                    def u_close(kb=kb, j=j, h1=h1, last=last, st=st):
                        if kb == 0:
                            nc.vector.tensor_copy(avacc[h1][:],
                                                  st["avps1"][:])
                        else:
                            nc.vector.tensor_add(avacc[h1][:], avacc[h1][:],
                                                 st["avps1"][:])
                        if last:
                            nc.sync.dma_start(out=l_dram[h1:h1 + 1, :],
                                              in_=avacc[h1][DK:DK + 1, :])
                            emit_half_tail(j, 1)

                    filler.append((False, False, u_open, "open"))
                    for kc in range(NKC):
                        filler.append((False, False,
                                       lambda kc=kc: u_av(kc)))
                    filler.append((False, False, u_close, "close"))
                    push_prefetch(kb, j)

            # ---------------- program ----------------
            # bo2 = bo + wo^T-contraction of bv, via filler units
            # (folds the V bias into the output-projection bias)
            bvb = wconst.tile([P, DCH], BF16, tag="bvb")
            bo2 = wconst.tile([P, DCH], F32, tag="bo2")

            def u_bo2(ec):
                if ec == 0:
                    nc.vector.tensor_copy(bvb[:], b_sb["bv"][:])
                bps = psP.tile([P, 1], F32, tag="s", name=f"bps{ec}")
                for dc in range(DCH):
                    nc.tensor.matmul(
                        bps[:], w_sb["wo"][:, dc, ec * P:(ec + 1) * P],
                        bvb[:, dc:dc + 1],
                        start=(dc == 0), stop=(dc == DCH - 1))
                nc.vector.tensor_scalar_add(bo2[:, ec:ec + 1], bps[:],
                                            b_sb["bo"][:, ec:ec + 1])

            # Per-pair prefetch pushes, balanced to the 8 psum-unit slots
            # each pair's drains provide (see drain() budgets).
            def push_prefetch(kb, j):
                if kb == 0:
                    if j == 0:
                        push_kproj(0, 2)
                        push_qproj(2)
                    elif j == 1:
                        push_kproj(0, 3)
                        push_qproj(3)
                elif kb == 1 and j == 1:
                    for ec in range(DCH):
                        filler.append((True, False,
                                       lambda ec=ec: u_bo2(ec)))
                # everything for kb+1 is queued by the end of pair 1 so
                # the filler runs dry before the block boundary and the
                # next block's first QK isn't stuck behind a clump of
                # projection matmuls in the PE queue
                if kb + 1 < NKB:
                    n = kb + 1
                    if j == 0:
                        push_dma(lambda n=n: emit_load_x(n))
                        push_dma(lambda n=n: emit_mask(n, range(0, 4)))
                        push_dma(lambda n=n: emit_mask(n, range(4, NKC)))
                        for ec in range(DCH):
                            push_kproj(n, ec)
                    elif j == 1:
                        push_vproj(n, range(NKC))

            # Upfront: just enough for pair 0 of kb0 to start immediately.
            # Junk matmuls (one tile, 8 rotating ranges, so the WAW deps
            # are deep enough to stream) trip the PE's HAM clock gate to
            # 8/8 during the x/weight DMA wait: the first projections
            # then run at 2.4GHz instead of 1.2.
            warm = wconst.tile([P, P], BF16, tag="warm")
            nc.gpsimd.memset(warm[:], 0.0)
            wps = psS.tile([P, SLOC], F32, tag="s", name="warmps")
            for i in range(56):
                c = (i % 8) * P
                nc.tensor.matmul(wps[:, c:c + P], warm[:], warm[:],
                                 start=True, stop=True)
            emit_load_x(0)
            emit_weight("wk")
            emit_bias("bk")
            emit_weight("wq")
            emit_bias("bq")
            emit_weight("wv")
            emit_bias("bv")
            for q2 in range(NQT):
                emit_kproj_unit(0, 0, q2)
            for q2 in range(NQT):
                emit_qproj_unit(0, q2)
            emit_vproj(0, [0, 1, 2, 3])
            emit_mask(0, range(NKC))
            emit_weight("wo")
            emit_bias("bo")
            # interleave pair-1's K/Q channel with kb0's remaining V chunks
            filler.append((True, False, lambda: emit_vproj(0, [4])))
            filler.append((True, False, lambda: emit_kproj_unit(0, 1, 0)))
            filler.append((True, False, lambda: emit_kproj_unit(0, 1, 1)))
            filler.append((True, False, lambda: emit_vproj(0, [5])))
            filler.append((True, False, lambda: emit_qproj_unit(1, 0)))
            filler.append((True, False, lambda: emit_qproj_unit(1, 1)))
            filler.append((True, False, lambda: emit_vproj(0, [6])))
            filler.append((True, False, lambda: emit_vproj(0, [7])))

            for kb in range(NKB):
                emit_attention(kb)

            # ---- tail: dense output projection, split per chunk into a
            # dc0-2 PSUM-held prefix and a dc3 finisher (dc3 = last head
            # pair, the only part that waits the final normalization).
            # The last pair's AV units are drained FIRST so the prefix
            # matmuls don't delay its accumulator adds.
            def oproj_pre(qt, ec):
                ops = psS.tile([P, QTS], F32, tag="s",
                               name=f"ops{ec}_{qt}")
                for dc in range(DCH - 1):
                    nc.tensor.matmul(
                        ops[:],
                        w_sb["wo"][:, dc, ec * P:(ec + 1) * P],
                        onorm[:, dc, qt * QTS:(qt + 1) * QTS],
                        start=(dc == 0), stop=False)
                return ops

            def oproj_post(qt, ec, ops):
                dc = DCH - 1
                nc.tensor.matmul(
                    ops[:],
                    w_sb["wo"][:, dc, ec * P:(ec + 1) * P],
                    onorm[:, dc, qt * QTS:(qt + 1) * QTS],
                    start=False, stop=True)
                oev = oep.tile([P, QTS], BF16, tag="oev",
                               name=f"oev{ec}_{qt}")
                nc.vector.tensor_scalar_add(oev[:], ops[:],
                                            bo2[:, ec:ec + 1])
                nc.sync.dma_start(
                    out=out[ec * P:(ec + 1) * P, qt * QTS:(qt + 1) * QTS],
                    in_=oev[:])

            while filler and (len(filler[0]) <= 3 or
                              filler[0][3] != "close"):
                filler.pop(0)[2]()
            pres = [(qt, ec, oproj_pre(qt, ec))
                    for qt, ec in [(0, 0), (1, 0), (0, 1)]]
            drain_all()  # u_close: h7's accumulator add + tail
            for qt, ec, ops in pres:
                oproj_post(qt, ec, ops)
            for qt, ec in [(1, 1), (0, 2), (1, 2), (0, 3), (1, 3)]:
                oproj_post(qt, ec, oproj_pre(qt, ec))
    nc.compile()
    return nc


def get_program():
    global _CACHED_NC
    if _CACHED_NC is None:
        _CACHED_NC = _build_program()
    return _CACHED_NC


def make_in_maps(x, dag_mask, wq, bq, wk, bk, wv, bv, wo, bo):
    """Host-side sharding: slices/transposes/rotations + bf16 casts only."""
    shared = {
        "wqT": np.ascontiguousarray(wq.T).astype(NPBF16),
        "wkT": np.ascontiguousarray(wk.T).astype(NPBF16),
        "wvT": np.ascontiguousarray(wv.T).astype(NPBF16),
        "woT": np.ascontiguousarray(wo.T).astype(NPBF16),
        "bq": np.ascontiguousarray(bq), "bk": np.ascontiguousarray(bk),
        "bv": np.ascontiguousarray(bv), "bo": np.ascontiguousarray(bo),
    }
    xTs = [np.ascontiguousarray(x[b].T).astype(NPBF16) for b in range(B)]
    mask_bf = dag_mask.astype(NPBF16)
    in_maps = []
    for c in range(NCORES):
        b, j = divmod(c, NCORES // B)
        s0 = j * SLOC
        # rotate the key axis so program block 0 == this core's query slice
        xTb = xTs[b]
        xT_rot = np.ascontiguousarray(
            np.concatenate([xTb[:, s0:], xTb[:, :s0]], axis=1))
        mT = mask_bf[s0:s0 + SLOC, :].T  # (S keys, SLOC queries)
        mT_rot = np.ascontiguousarray(
            np.concatenate([mT[s0:, :], mT[:s0, :]], axis=0))
        in_maps.append({"xT": xT_rot, "maskT": mT_rot, **shared})
    return in_maps


def kernel(x, dag_mask, wq, bq, wk, bk, wv, bv, wo, bo, trace=False):
    x = np.asarray(x, dtype=np.float32)
    dag_mask = np.asarray(dag_mask, dtype=np.int32)
    args = [np.asarray(a, dtype=np.float32)
            for a in (wq, bq, wk, bk, wv, bv, wo, bo)]
    nc = get_program()
    in_maps = make_in_maps(x, dag_mask, *args)
    core_ids = list(range(NCORES))
    res = run_bass_kernel_spmd(nc, in_maps, core_ids, trace=trace)
    out = np.empty((B, S, DM), np.float32)
    for c in range(NCORES):
        b, j = divmod(c, NCORES // B)
        s0 = j * SLOC
        out[b, s0:s0 + SLOC, :] = res.results[c]["out"].T.astype(np.float32)
    if trace:
        return out, res
    return out

